# revision 1
# baseline (speedup 1.0000x reference)
"""Trainium2 Bass kernel for nn_CPE_47364899340506 (submanifold sparse 3D conv).

Reference semantics: coords quantized from depth onto a 65^3 voxel grid, a
global voxel->point-index map (max-index dedup), then for each of 27 kernel
offsets gather active-neighbor features and GEMM with the per-offset
[256, 256] weight, accumulating over offsets.

Strategy (8 NeuronCores, SPMD, full inputs in / full output out):
  Host (numpy integer work, bit-exact replica of the reference index math):
    - shard the 65552 points by image row-band (8 rows per core), voxel-sorted
      within each core;
    - per core, per group of ~9 point-tiles, build a compact voxel-sorted
      "winner" feature table; because the table is voxel-sorted, the three
      dz in {-1,0,1} taps of any (point, direction) triple always hit
      CONSECUTIVE table rows, so each triple is served by ONE 1.5KB gather
      descriptor from a pattern-region table (valid/invalid combos encoded as
      contiguous 3-unit patterns: plain run / [0,X,0,X..] / [0,0,X_m,X_m+1]
      blocks).  This cuts DMA descriptors 3x vs per-tap gathering - the
      SDMA descriptor rate (~9 ns/descriptor aggregate) is the bottleneck.
  Device (per core): for each 128-point tile, one dma_gather(transpose=True)
  (1152 descriptors, 1.5KB each) lands all 27 taps' neighbor features in
  [ci, pt] layout; 54 back-to-back fp16 matmuls (27 taps x 2 ci-chunks)
  accumulate the [128, 256] output tile in fp32 PSUM.
"""
import itertools
from contextlib import ExitStack

import numpy as np

BND = 64
G = BND + 1
B, H, W, C = 16, 64, 64, 256
HW = H * W
N = B * (HW + 1)              # 65552
NCORES = 8
NLOC = N // NCORES            # 8194
P = 128
NT = (NLOC + P - 1) // P      # 65 tiles (last has 2 live rows)
TAPS = 27
CHUNKS = 2
TRI_NIDX = 9 * P              # one 3-tap descriptor per (point, direction)
TRI_S = TRI_NIDX // 16
TILE_GRP = 9                  # tiles per winner-table group
NGRP = (NT + TILE_GRP - 1) // TILE_GRP
OFFSETS = np.array(list(itertools.product([-1, 0, 1], repeat=3)), dtype=np.int32)

_COMPILED = {}


# ---------------------------------------------------------------- host prep --

def _compute_coords(depth):
    ah = np.arange(H, dtype=np.float32) / np.float32(H - 1)
    aw = np.arange(W, dtype=np.float32) / np.float32(W - 1)
    y, x = np.meshgrid(ah, aw, indexing="ij")
    zmin = depth.min(axis=(1, 2), keepdims=True)
    zmax = depth.max(axis=(1, 2), keepdims=True)
    z = (depth - zmin) / (zmax - zmin + np.float32(1e-8))
    bx = np.broadcast_to(x, (B, H, W)).astype(np.float32)
    by = np.broadcast_to(y, (B, H, W)).astype(np.float32)
    coords = np.stack([bx, by, z], axis=-1)
    coord = coords.reshape(B, HW, 3)
    coord = np.clip(np.round(coord * np.float32(BND)), 0, BND).astype(np.int32)
    cls = np.zeros((B, 1, 3), dtype=np.int32)
    return np.concatenate([cls, coord], axis=1).reshape(-1, 3)


def _compute_nid_valid(coord):
    lin = (coord[:, 0] * G + coord[:, 1]) * G + coord[:, 2]
    idx_map = np.full((G * G * G,), -1, dtype=np.int32)
    np.maximum.at(idx_map, lin, np.arange(N, dtype=np.int32))
    nb = coord[None, :, :] + OFFSETS[:, None, :]
    inb = np.all((nb >= 0) & (nb <= BND), axis=-1)
    nbc = np.clip(nb, 0, BND)
    nlin = (nbc[..., 0] * G + nbc[..., 1]) * G + nbc[..., 2]
    nid = idx_map[nlin]
    valid = inb & (nid >= 0)
    return nid, valid


def _core_point_assignment(coord):
    idx = np.arange(N, dtype=np.int32)
    rel = idx % (HW + 1)
    batch = idx // (HW + 1)
    is_cls = rel == 0
    row = (rel - 1) // W
    band = np.where(is_cls, batch // 2, row // 8)
    order = np.argsort(band, kind="stable").astype(np.int32)
    perm = order.reshape(NCORES, NLOC)
    voxkey = (coord[:, 1].astype(np.int64) * G + coord[:, 0]) * G + coord[:, 2]
    return np.stack([p[np.argsort(voxkey[p], kind="stable")] for p in perm])


def _units_total(rows):
    q2 = rows + 2
    q3 = q2 + 4 + 2 * (rows + 1) + 2 + 4   # [X_0,0,0,0] pad block at q3-4
    return q3 + 4 * (rows + 1) + 4, q2, q3


def _build_triple_tables(features, coord, nid, valid, perm):
    voxkey = (coord[:, 1].astype(np.int64) * G + coord[:, 0]) * G + coord[:, 2]
    per_core = []
    max_rows = 0
    for c in range(NCORES):
        pts_all = perm[c]
        groups = []
        for g in range(NGRP):
            t0, t1 = g * TILE_GRP, min((g + 1) * TILE_GRP, NT)
            p0, p1 = t0 * P, min(t1 * P, NLOC)
            pts = pts_all[p0:p1]
            nid_g = nid[:, pts]
            val_g = valid[:, pts]
            used = np.unique(nid_g[val_g])
            used = used[np.argsort(voxkey[used], kind="stable")]
            rows = len(used)
            max_rows = max(max_rows, rows)
            gid_order = np.argsort(used)
            gid_sorted = used[gid_order]

            def lookup(garr):
                return gid_order[np.searchsorted(gid_sorted, garr)]

            tot, q2, q3 = _units_total(rows)
            npts = p1 - p0
            units = np.zeros((9, npts), dtype=np.int64)
            for d in range(9):
                k0, k1, k2 = d * 3, d * 3 + 1, d * 3 + 2
                v1, v2, v3 = val_g[k0], val_g[k1], val_g[k2]
                r1 = np.where(v1, lookup(np.where(v1, nid_g[k0], used[0])), -1)
                r2 = np.where(v2, lookup(np.where(v2, nid_g[k1], used[0])), -1)
                r3 = np.where(v3, lookup(np.where(v3, nid_g[k2], used[0])), -1)
                combo = v1.astype(np.int64) * 4 + v2 * 2 + v3
                u = np.full(npts, q2, dtype=np.int64)          # (i,i,i)
                u = np.where(combo == 7, r1, u)                # (v,v,v)
                np.testing.assert_array_equal(r2[combo == 7], r1[combo == 7] + 1)
                np.testing.assert_array_equal(r3[combo == 7], r1[combo == 7] + 2)
                u = np.where(combo == 2, q2 + 4 + 2 * r2, u)   # (i,v,i)
                u = np.where(combo == 5, q2 + 5 + 2 * r1, u)   # (v,i,v)
                np.testing.assert_array_equal(r3[combo == 5], r1[combo == 5] + 1)
                u = np.where(combo == 1, q3 + 4 * r3, u)       # (i,i,v)
                u = np.where(combo == 3, q3 + 4 * r2 + 1, u)   # (i,v,v)
                np.testing.assert_array_equal(r3[combo == 3], r2[combo == 3] + 1)
                u = np.where(combo == 6, q3 + 4 * r1 + 2, u)   # (v,v,i)
                np.testing.assert_array_equal(r2[combo == 6], r1[combo == 6] + 1)
                u = np.where(combo == 4,                       # (v,i,i)
                             np.where(r1 == 0, q3 - 4, q3 + 4 * r1 - 1), u)
                units[d] = u
            groups.append((used, units))
        per_core.append(groups)
    u_tot_max, _, _ = _units_total(max_rows)
    u_sub = ((u_tot_max + 127) // 128) * 128
    assert u_sub <= 32640, f"triple table too large for int16: {u_sub}"

    mega = np.zeros((NCORES, NGRP, u_sub, C), dtype=np.float16)
    idxw = np.zeros((NCORES, P, NT * TRI_S), dtype=np.int16)
    for c in range(NCORES):
        units_full = np.zeros((9, NT * P), dtype=np.int64)
        for g, (used, units) in enumerate(per_core[c]):
            rows = len(used)
            _, q2, q3 = _units_total(rows)
            X = features[used].astype(np.float16)
            m = mega[c, g]
            m[0:rows] = X
            m[q2 + 5 + 2 * np.arange(rows)] = X
            m[q3 - 4] = X[0]
            m[q3 + 4 * np.arange(rows) + 2] = X
            if rows > 1:
                m[q3 + 4 * np.arange(rows - 1) + 3] = X[1:]
            p0 = g * TILE_GRP * P
            units_full[:, p0:p0 + units.shape[1]] = units
            pend = min((g + 1) * TILE_GRP, NT) * P
            if pend > p0 + units.shape[1]:
                units_full[:, p0 + units.shape[1]:pend] = q2
        ua = units_full.reshape(9, NT, P)
        out = np.zeros((NT, TRI_NIDX), dtype=np.int64)
        for d in range(9):
            out[:, d * P:(d + 1) * P] = ua[d]
        wrapped = out.reshape(NT, TRI_S, 16).transpose(0, 2, 1)
        wrapped = np.tile(wrapped, (1, 8, 1))
        idxw[c] = wrapped.transpose(1, 0, 2).reshape(P, NT * TRI_S)
    return mega, idxw, u_sub


def _build_weight_input(weight):
    w = weight.astype(np.float16).reshape(TAPS, CHUNKS, P, C)
    return np.ascontiguousarray(w.transpose(2, 0, 1, 3).reshape(P, TAPS * CHUNKS * C))


# ------------------------------------------------------------- device kernel --

def _build_bass(u_sub):
    import concourse.bacc as bacc
    import concourse.bass as bass
    import concourse.tile as tile
    from concourse import mybir

    F16, F32, I16 = mybir.dt.float16, mybir.dt.float32, mybir.dt.int16
    nc = bacc.Bacc("TRN2", target_bir_lowering=False, debug=False,
                   num_devices=NCORES, dynamic_dma_scratch_size=65536)
    mega = nc.dram_tensor("mega", [NGRP * u_sub, C], F16, kind="ExternalInput").ap()
    idx = nc.dram_tensor("idx", [P, NT * TRI_S], I16, kind="ExternalInput").ap()
    wts = nc.dram_tensor("wts", [P, TAPS * CHUNKS * C], F16, kind="ExternalInput").ap()
    out = nc.dram_tensor("out", [NLOC, C], F32, kind="ExternalOutput").ap()

    with tile.TileContext(nc) as tc, ExitStack() as ctx:
        const_pool = ctx.enter_context(tc.tile_pool(name="const", bufs=1))
        gpool = ctx.enter_context(tc.tile_pool(name="gather", bufs=3))
        pspool = ctx.enter_context(tc.tile_pool(name="psum", bufs=4, space="PSUM"))
        opool = ctx.enter_context(tc.tile_pool(name="outp", bufs=3))

        w_tile = const_pool.tile([P, TAPS * CHUNKS * C], F16, tag="wts")
        nc.sync.dma_start(out=w_tile[:], in_=wts[:])
        idx_tile = const_pool.tile([P, NT * TRI_S], I16, tag="idx")
        nc.sync.dma_start(out=idx_tile[:], in_=idx[:])

        for t in range(NT):
            g = t // TILE_GRP
            src = bass.AP(mega.tensor, g * u_sub * C, [[C, u_sub - 2], [1, 768]])
            gt = gpool.tile([P, 6, TRI_NIDX], F16, tag="g")
            nc.gpsimd.dma_gather(
                out_ap=gt[:, :, :],
                in_ap=src,
                idxs_ap=idx_tile[:, t * TRI_S:(t + 1) * TRI_S],
                num_idxs=TRI_NIDX,
                num_idxs_reg=TRI_NIDX,
                elem_size=768,
                elem_step=C,
                transpose=True,
                single_packet=False,
            )
            ps = pspool.tile([P, C], F32)
            i_mm = 0
            for d in range(9):
                for dzi in range(3):
                    k = d * 3 + dzi
                    for cc in range(CHUNKS):
                        nc.tensor.matmul(
                            ps[:, :],
                            lhsT=gt[:, dzi * 2 + cc, d * P:(d + 1) * P],
                            rhs=w_tile[:, (k * CHUNKS + cc) * C:(k * CHUNKS + cc + 1) * C],
                            start=(i_mm == 0),
                            stop=(i_mm == TAPS * CHUNKS - 1),
                        )
                        i_mm += 1
            o = opool.tile([P, C], F32)
            nc.vector.tensor_copy(o[:, :], ps[:, :])
            rows = min(P, NLOC - t * P)
            nc.sync.dma_start(out=out[t * P:t * P + rows, :], in_=o[:rows, :])
    nc.compile()
    return nc


# --------------------------------------------------------------- entry point --

def kernel(features, depth, weight):
    from concourse.bass_utils import run_bass_kernel_spmd

    features = np.asarray(features, dtype=np.float32)
    depth = np.asarray(depth, dtype=np.float32)
    weight = np.asarray(weight, dtype=np.float32)

    coord = _compute_coords(depth)
    nid, valid = _compute_nid_valid(coord)
    perm = _core_point_assignment(coord)
    mega, idxw, u_sub = _build_triple_tables(features, coord, nid, valid, perm)
    w_dev = _build_weight_input(weight)

    if u_sub not in _COMPILED:
        _COMPILED[u_sub] = _build_bass(u_sub)
    nc = _COMPILED[u_sub]

    in_maps = [{"mega": mega[c].reshape(-1, C), "idx": idxw[c], "wts": w_dev}
               for c in range(NCORES)]
    res = run_bass_kernel_spmd(nc, in_maps, list(range(NCORES)))

    out = np.empty((N, C), dtype=np.float32)
    for c in range(NCORES):
        out[perm[c]] = res.results[c]["out"]
    return out



# revision 13
# speedup vs baseline: 1.8556x; 1.8556x over previous
"""Trainium2 Bass kernel for nn_CPE_47364899340506 (submanifold sparse 3D conv).

Reference semantics: coords quantized from depth onto a 65^3 voxel grid, a
global voxel->point-index map (max-index dedup), then for each of 27 kernel
offsets gather active-neighbor features and GEMM with the per-offset
[256, 256] weight, accumulating over offsets.

Strategy (8 NeuronCores, SPMD, full inputs in / full output out):
  The microbenchmarked DMA cost is ~130 ns fixed per gather descriptor
  (regardless of source being HBM or SBUF or of payload below ~1.5KB) plus
  ~24 B/ns marginal; the PE floor for the dense 27-tap GEMM is ~6.3 us per
  128-point tile.  The v1 kernel's 1152 x 1.5KB descriptors/tile (~9.5 us)
  were therefore descriptor-fixed-cost bound.

  v2: the host lays out, per (tile, direction, group-of-8-points), one
  contiguous 12 KB block holding the 8 points' 3 dz-tap neighbor features
  (zeros where invalid).  The device gather then needs only 144 descriptors
  x 12 KB per tile (~5.2 us, below the PE floor), each landing 48 transpose
  sub-rows in SBUF.  The 54 accumulating matmuls read the stationary operand
  through a 2D-strided access pattern [[1536,8],[1,16]] that walks
  (member, group) so PSUM partition s corresponds to point slot s.
  num_idxs is padded 144->256 with trailing -1 indices, which the SWDGE Q7
  kernel trims before descriptor generation (free).
"""
import itertools
from contextlib import ExitStack

import numpy as np

BND = 64
G = BND + 1
B, H, W, C = 16, 64, 64, 256
HW = H * W
N = B * (HW + 1)              # 65552
NCORES = 8
NLOC = N // NCORES            # 8194
P = 128
NT = (NLOC + P - 1) // P      # 65 tiles (last has 2 live rows)
TAPS = 27
CHUNKS = 2
NDESC = TAPS * CHUNKS         # 54 live descriptors per (tile, half): (d,dzi,cc)
NIDX = 128                    # padded to %128==0; trailing -1 trimmed by HW
BLK_ELEMS = P * P // 2        # 8192 elements (16 KB): 64 point slots x 128 ci
SUBS = P                      # transpose sub-rows across both halves (sub=point)
NBLK = NT * NDESC * 2         # 7020 mega half-blocks per core
OFFSETS = np.array(list(itertools.product([-1, 0, 1], repeat=3)), dtype=np.int32)

_COMPILED = {}


# ---------------------------------------------------------------- host prep --

def _compute_coords(depth):
    ah = np.arange(H, dtype=np.float32) / np.float32(H - 1)
    aw = np.arange(W, dtype=np.float32) / np.float32(W - 1)
    y, x = np.meshgrid(ah, aw, indexing="ij")
    zmin = depth.min(axis=(1, 2), keepdims=True)
    zmax = depth.max(axis=(1, 2), keepdims=True)
    z = (depth - zmin) / (zmax - zmin + np.float32(1e-8))
    bx = np.broadcast_to(x, (B, H, W)).astype(np.float32)
    by = np.broadcast_to(y, (B, H, W)).astype(np.float32)
    coords = np.stack([bx, by, z], axis=-1)
    coord = coords.reshape(B, HW, 3)
    coord = np.clip(np.round(coord * np.float32(BND)), 0, BND).astype(np.int32)
    cls = np.zeros((B, 1, 3), dtype=np.int32)
    return np.concatenate([cls, coord], axis=1).reshape(-1, 3)


def _compute_nid_valid(coord):
    lin = (coord[:, 0] * G + coord[:, 1]) * G + coord[:, 2]
    idx_map = np.full((G * G * G,), -1, dtype=np.int32)
    np.maximum.at(idx_map, lin, np.arange(N, dtype=np.int32))
    nb = coord[None, :, :] + OFFSETS[:, None, :]
    inb = np.all((nb >= 0) & (nb <= BND), axis=-1)
    nbc = np.clip(nb, 0, BND)
    nlin = (nbc[..., 0] * G + nbc[..., 1]) * G + nbc[..., 2]
    nid = idx_map[nlin]
    valid = inb & (nid >= 0)
    return nid, valid


def _core_point_assignment(coord):
    idx = np.arange(N, dtype=np.int32)
    rel = idx % (HW + 1)
    batch = idx // (HW + 1)
    is_cls = rel == 0
    row = (rel - 1) // W
    band = np.where(is_cls, batch // 2, row // 8)
    order = np.argsort(band, kind="stable").astype(np.int32)
    perm = order.reshape(NCORES, NLOC)
    voxkey = (coord[:, 1].astype(np.int64) * G + coord[:, 0]) * G + coord[:, 2]
    return np.stack([p[np.argsort(voxkey[p], kind="stable")] for p in perm])


def _build_mega(features, nid, valid, perm):
    """[NCORES, NBLK, BLK_ELEMS] fp16.  Block (t, d, dzi, cc) holds the
    tile's 128 point slots x 128 ci-chunk neighbor features for tap
    k=d*3+dzi, zeros where invalid.  Element e = p*128 + ci', so after the
    transpose gather partition=ci', sub=p, and the matmul stationary AP is
    the plain 1-free-dim slice gt[:, :, col]."""
    f16 = features.astype(np.float16)
    feat_ext = np.concatenate([f16, np.zeros((1, C), np.float16)], axis=0)
    # masked neighbor row per (tap, point): N = the zero row
    nid_masked = np.where(valid, nid, N).astype(np.int32)
    # sentinel column for dead slots
    nidp = np.concatenate([nid_masked, np.full((TAPS, 1), N, np.int32)], axis=1)

    mega = np.empty((NCORES, NBLK, BLK_ELEMS), dtype=np.float16)
    for c in range(NCORES):
        pts = np.full(NT * P, N, np.int32)
        pts[:NLOC] = perm[c]
        g = nidp[:, pts.reshape(NT, P)]          # [27, NT, p]
        rows = feat_ext[g.reshape(-1)]           # [27*NT*128, 256]
        rows = rows.reshape(TAPS, NT, 2, P // 2, CHUNKS, P)
        # [k, t, h, p', cc, ci'] -> [t, k, cc, h, p', ci']
        mega[c] = np.ascontiguousarray(
            rows.transpose(1, 0, 4, 2, 3, 5)).reshape(NBLK, BLK_ELEMS)
    return mega


NPAIR = NT // 2               # 32 full tile-pairs; tile 64 handled alone
NGATH = NPAIR * 2 + 2         # gathers per core: (pair, h) + odd tile's two
LIVE_PAIR = 2 * NDESC + 4     # 108 real + 4 dummy repeats -> %16 == 0
LIVE_ODD = NDESC + 10         # 54 real + 10 dummy repeats -> 64


def _build_idx():
    """Gather index tables, identical for every core: [128, NGATH*NIDX//16]
    i16.  Gather (pair p, half h) covers tiles 2p,2p+1: positions 0..53 ->
    half-blocks of tile 2p, 54..107 -> tile 2p+1, then dummy repeats up to
    the %16 live count (num_idxs_reg), then -1 (trimmed by the Q7 gen)."""
    j = np.arange(NDESC)
    lists = []
    for p in range(NPAIR):
        for h in range(2):
            row = np.full(NIDX, -1, np.int64)
            row[:NDESC] = ((2 * p) * NDESC + j) * 2 + h
            row[NDESC:2 * NDESC] = ((2 * p + 1) * NDESC + j) * 2 + h
            row[2 * NDESC:LIVE_PAIR] = row[2 * NDESC - 1]
            lists.append(row)
    for h in range(2):
        row = np.full(NIDX, -1, np.int64)
        row[:NDESC] = ((NT - 1) * NDESC + j) * 2 + h
        row[NDESC:LIVE_ODD] = row[NDESC - 1]
        lists.append(row)
    out = np.stack(lists)                      # [NGATH, NIDX]
    # wrap: position j -> partition j%16, column j//16
    wrapped = out.reshape(NGATH, NIDX // 16, 16).transpose(0, 2, 1)
    wrapped = np.tile(wrapped, (1, 8, 1))
    return np.ascontiguousarray(
        wrapped.transpose(1, 0, 2).reshape(128, NGATH * (NIDX // 16))).astype(np.int16)


def _build_weight_input(weight):
    w = weight.astype(np.float16).reshape(TAPS, CHUNKS, P, C)
    return np.ascontiguousarray(w.transpose(2, 0, 1, 3).reshape(P, TAPS * CHUNKS * C))


def prepare_inputs(features, depth, weight):
    coord = _compute_coords(depth)
    nid, valid = _compute_nid_valid(coord)
    perm = _core_point_assignment(coord)
    mega = _build_mega(features, nid, valid, perm)
    idxw = _build_idx()
    w_dev = _build_weight_input(weight)
    in_maps = [{"mega": mega[c], "idx": idxw, "wts": w_dev}
               for c in range(NCORES)]
    return in_maps, perm


# ------------------------------------------------------------- device kernel --

def _build_bass():
    import concourse.bacc as bacc
    import concourse.bass as bass
    import concourse.tile as tile
    from concourse import mybir

    F16, F32, I16 = mybir.dt.float16, mybir.dt.float32, mybir.dt.int16
    S16 = NIDX // 16
    nc = bacc.Bacc("TRN2", target_bir_lowering=False, debug=False,
                   num_devices=NCORES, dynamic_dma_scratch_size=65536)
    mega = nc.dram_tensor("mega", [NBLK, BLK_ELEMS], F16, kind="ExternalInput").ap()
    idx = nc.dram_tensor("idx", [P, NGATH * S16], I16, kind="ExternalInput").ap()
    wts = nc.dram_tensor("wts", [P, TAPS * CHUNKS * C], F16, kind="ExternalInput").ap()
    out = nc.dram_tensor("out", [NLOC, C], F32, kind="ExternalOutput").ap()

    with tile.TileContext(nc) as tc, ExitStack() as ctx:
        const_pool = ctx.enter_context(tc.tile_pool(name="const", bufs=1))
        gpool = ctx.enter_context(tc.tile_pool(name="gather", bufs=3))
        pspool = ctx.enter_context(tc.tile_pool(name="psum", bufs=4, space="PSUM"))
        opool = ctx.enter_context(tc.tile_pool(name="outp", bufs=3))

        w_tile = const_pool.tile([P, TAPS * CHUNKS * C], F16, tag="wts")
        nc.sync.dma_start(out=w_tile[:], in_=wts[:])
        idx_tile = const_pool.tile([P, NGATH * S16], I16, tag="idx")
        nc.sync.dma_start(out=idx_tile[:], in_=idx[:])

        src = bass.AP(mega.tensor, 0, [[BLK_ELEMS, NBLK], [1, BLK_ELEMS]])
        for p in range(NPAIR + 1):
            odd = p == NPAIR
            live = LIVE_ODD if odd else LIVE_PAIR
            gt = gpool.tile([P, SUBS, NIDX], F16, tag="g")
            for h in range(2):
                g_i = 2 * p + h
                nc.gpsimd.dma_gather(
                    out_ap=gt[:, h * (P // 2):(h + 1) * (P // 2), :],
                    in_ap=src,
                    idxs_ap=idx_tile[:, g_i * S16:(g_i + 1) * S16],
                    num_idxs=NIDX,
                    num_idxs_reg=live,
                    elem_size=BLK_ELEMS,
                    elem_step=BLK_ELEMS,
                    transpose=True,
                    single_packet=False,
                )
            for q in range(1 if odd else 2):
                t = 2 * p + q
                ps = pspool.tile([P, C], F32)
                for i_mm in range(NDESC):
                    nc.tensor.matmul(
                        ps[:, :],
                        lhsT=gt[:, :, q * NDESC + i_mm],
                        rhs=w_tile[:, i_mm * C:(i_mm + 1) * C],
                        start=(i_mm == 0),
                        stop=(i_mm == NDESC - 1),
                    )
                o = opool.tile([P, C], F32)
                nc.vector.tensor_copy(o[:, :], ps[:, :])
                rows = min(P, NLOC - t * P)
                nc.sync.dma_start(out=out[t * P:t * P + rows, :], in_=o[:rows, :])
    nc.compile()
    return nc


# --------------------------------------------------------------- entry point --

def kernel(features, depth, weight):
    from concourse.bass_utils import run_bass_kernel_spmd

    features = np.asarray(features, dtype=np.float32)
    depth = np.asarray(depth, dtype=np.float32)
    weight = np.asarray(weight, dtype=np.float32)

    in_maps, perm = prepare_inputs(features, depth, weight)

    if "v2" not in _COMPILED:
        _COMPILED["v2"] = _build_bass()
    nc = _COMPILED["v2"]

    res = run_bass_kernel_spmd(nc, in_maps, list(range(NCORES)))

    out = np.empty((N, C), dtype=np.float32)
    for c in range(NCORES):
        out[perm[c]] = res.results[c]["out"]
    return out
